# revision 11
# baseline (speedup 1.0000x reference)
"""Tensor-parallel causal attention (MQA, partial NeoX RoPE) on 8 TRN2 NeuronCores.

Sharding (tensor-parallel over heads, as in the original module):
  core c owns query heads [16c, 16c+16) (rows of Wq), kv head c (rows of Wkv),
  and columns [1024c, 1024(c+1)) of Wo.  Attention is embarrassingly parallel
  per head group; the dense output projection produces per-core partials that
  are combined with on-device ReduceScatters (token-sharded per batch half,
  feature-chunked so the collectives overlap the tail of the projection).
  The host reassembles the 8 disjoint token shards.

Per-core dataflow (all matmuls bf16, accumulation + softmax math in f32):
  phase 0: constants (identity, triangular mask, rope cos/sin).
  phase 1: per 512-token chunk: DVE-cast hs to bf16, PE-transpose (bf16 --
           4x faster than the fp32 LOW_HIGH path) into hsT; Q projection
           (bf16 copy of Wq materialized in DRAM once, streamed per chunk)
           and K/V projection, evicted through fused RoPE into qT/kT (bf16)
           and vT; v re-transposed into vext with a ones-column.
  phase 2: per (batch, i-chunk, head pair): scoresT[j,i] = kT.T @ qT with the
           column range truncated to the causal triangle; exp on ACT
           (scale=1/8 folded in); single [128,128] triangular mask multiply
           on the diagonal sub-block (DVE); PV matmul with the ones-column
           giving softmax denominators for free.  Scores for j-tile t+1 are
           emitted before PV of j-tile t so the exp latency never stalls PE.
           Denominators for all 16 heads are gathered into one [16,512] tile,
           inverted with one reciprocal_approx_fast, and applied in place.
  phase 3: out[t,o] = attnT.T @ WoT per batch half, o-chunked; Wo streamed
           f32 and cast on DVE; each [1024, 2048] bf16 partial fires a
           ReduceScatter; final copies to the external output run at the end
           on the gpsimd queue.
"""

import math
from dataclasses import dataclass

import numpy as np


# ---------------------------------------------------------------- config

@dataclass(frozen=True)
class Cfg:
    n_cores: int = 8
    T: int = 2048          # total tokens (B*S)
    B: int = 2             # sequences
    HID: int = 8192        # hidden size (= total heads * D)
    MQ: int = 1024         # per-core query dims (16 heads * 64)
    D: int = 64            # head dim
    RD: int = 32           # rotary dims (first RD of each head)
    CHUNK: int = 512       # phase-1 token chunk
    IC: int = 512          # attention i-chunk width

    @property
    def S(self):
        return self.T // self.B

    @property
    def KT(self):
        return self.HID // 128   # k-tiles

    @property
    def MT(self):
        return self.MQ // 128    # per-core q-dim tiles (2 heads per tile)


FULL = Cfg()


# ---------------------------------------------------------------- builder

def build_nc(cfg: Cfg, enable_asserts: bool = False, debug: bool = False,
             truncate: bool = True, pipelined: bool = True):
    import concourse.bass as bass
    import concourse.mybir as mybir
    import concourse.tile as tile
    from concourse import bacc
    from concourse.masks import make_identity

    f32 = mybir.dt.float32
    bf16 = mybir.dt.bfloat16
    i32 = mybir.dt.int32

    T, HID, MQ, D, RD = cfg.T, cfg.HID, cfg.MQ, cfg.D, cfg.RD
    B, S = cfg.B, cfg.S
    CHUNK, IC = cfg.CHUNK, cfg.IC
    KT, MT = cfg.KT, cfg.MT
    NCH = T // CHUNK
    TSUB = CHUNK // 128
    HALF = RD // 2                      # 16
    SCALE = 1.0 / math.sqrt(D)
    NJ = 128                            # j-tile width
    NHP = MQ // D // 2                  # head pairs (8)
    TOUT = T // cfg.n_cores             # output rows per core
    FW = 2048                           # reduce-scatter feature-chunk width
    NF = HID // FW                      # feature chunks (4)

    nc = bacc.Bacc(
        "TRN2",
        target_bir_lowering=False,
        debug=debug,
        enable_asserts=enable_asserts,
        num_devices=cfg.n_cores,
    )

    hs_ext = nc.dram_tensor("hs", [T, HID], f32, kind="ExternalInput").ap()
    wq_ext = nc.dram_tensor("wqT", [HID, MQ], f32, kind="ExternalInput").ap()
    wkv_ext = nc.dram_tensor("wkv", [2 * D, HID], f32, kind="ExternalInput").ap()
    wo_ext = nc.dram_tensor("woT", [MQ, HID], f32, kind="ExternalInput").ap()
    pos_ext = nc.dram_tensor("pos", [1, T], f32, kind="ExternalInput").ap()
    out_ext = nc.dram_tensor("out", [TOUT, HID], f32, kind="ExternalOutput").ap()

    groups = [list(range(cfg.n_cores))]

    with tile.TileContext(nc) as tc:
        with (
            tc.tile_pool(name="const", bufs=1) as const_pool,
            tc.tile_pool(name="persist", bufs=1) as pp,
            tc.tile_pool(name="dram", bufs=1, space="DRAM") as dram,
        ):
            # ---- constants ------------------------------------------------
            identity = const_pool.tile([128, 128], bf16)
            make_identity(nc, identity)

            # triangular mask for diagonal 128x128 sub-blocks:
            # keep where j-offset (partition p) <= i-offset (free f)
            tri_mask = const_pool.tile([128, 128], bf16, name="tri")
            nc.gpsimd.memset(tri_mask, 1.0)
            nc.gpsimd.affine_select(
                out=tri_mask, in_=tri_mask,
                compare_op=mybir.AluOpType.is_ge,
                fill=0.0, base=0,
                pattern=[[1, 128]], channel_multiplier=-1,
            )
            full_masks = []
            if not truncate:
                for di in range(IC // 128):
                    mk = const_pool.tile([128, IC], bf16, name=f"mask{di}")
                    nc.gpsimd.memset(mk, 1.0)
                    nc.gpsimd.affine_select(
                        out=mk, in_=mk,
                        compare_op=mybir.AluOpType.is_ge,
                        fill=0.0, base=-di * 128,
                        pattern=[[1, IC]], channel_multiplier=-1,
                    )
                    full_masks.append(mk)

            # rope tables: cosT/sinT [HALF, T] f32
            iota_i = const_pool.tile([HALF, 1], i32)
            nc.gpsimd.iota(iota_i, pattern=[[1, 1]], base=0, channel_multiplier=1)
            iota_f = const_pool.tile([HALF, 1], f32)
            nc.vector.tensor_copy(iota_f, iota_i)
            invf = const_pool.tile([HALF, 1], f32)
            nc.scalar.activation(
                invf, iota_f, mybir.ActivationFunctionType.Exp,
                scale=-math.log(10000.0) / HALF,
            )
            sinT = const_pool.tile([HALF, T], f32)
            cosT = const_pool.tile([HALF, T], f32)
            twopi = 2.0 * math.pi
            c1 = 6.28125
            c2 = float(np.float32(twopi - c1))
            c3 = twopi - c1 - float(c2)
            with tc.tile_pool(name="ropetmp", bufs=1) as rtp:
                pos_sb = rtp.tile([1, T], f32)
                nc.sync.dma_start(out=pos_sb, in_=pos_ext)
                posb = rtp.tile([HALF, T], f32)
                nc.gpsimd.partition_broadcast(posb, pos_sb)
                freqT = rtp.tile([HALF, T], f32)
                nc.vector.tensor_scalar_mul(freqT, posb, invf)
                # range-reduce freqs into (-pi, pi] before ScalarE Sin
                kf = rtp.tile([HALF, T], f32)
                nc.vector.tensor_scalar_mul(kf, freqT, 1.0 / twopi)
                ki = rtp.tile([HALF, T], i32)
                nc.vector.tensor_copy(ki, kf)
                nc.vector.tensor_copy(kf, ki)
                red = rtp.tile([HALF, T], f32)
                nc.vector.cody_waite_cascade(red, freqT, kf, c1, c2, c3)
                sarg = rtp.tile([HALF, T], f32)
                nc.vector.add_range_wrap(sarg, red, 0.0, math.pi, twopi)
                nc.scalar.activation(
                    sinT, sarg, mybir.ActivationFunctionType.Sin
                )
                carg = rtp.tile([HALF, T], f32)
                nc.vector.add_range_wrap(carg, red, math.pi / 2, math.pi, twopi)
                nc.scalar.activation(
                    cosT, carg, mybir.ActivationFunctionType.Sin
                )

            # ---- persistent activations ----------------------------------
            qT = pp.tile([128, MT, T], bf16)          # q-dim-major, rope'd
            # k replicated in both partition halves so scores matmuls can
            # align lhsT/rhs base partitions for odd heads
            kT2 = pp.tile([128, T], bf16)
            vT = pp.tile([64, T], bf16)
            vext = pp.tile([128, T // 128, D + 1], bf16)
            nc.vector.memset(vext[:, :, D:D + 1], 1.0)

            # k-major bf16 copy of wqT, materialized row-block-wise
            wq_bf = dram.tile([HID, MQ], bf16, name="wq_bf")

            # reduce-scatter buffers: per (batch, feature chunk); the last
            # (b=1) feature chunk is split in two to shorten the tail
            rs_widths = []
            for b in range(B):
                for f in range(NF):
                    if b == B - 1 and f == NF - 1:
                        rs_widths += [(b, f * FW, FW // 2),
                                      (b, f * FW + FW // 2, FW // 2)]
                    else:
                        rs_widths.append((b, f * FW, FW))
            partials = [
                dram.tile([S, w], bf16, name=f"partial{i}")
                for i, (b, o0, w) in enumerate(rs_widths)
            ]
            rs_outs = [
                dram.tile([S // cfg.n_cores, w], bf16, name=f"rs_out{i}")
                for i, (b, o0, w) in enumerate(rs_widths)
            ]

            # ---- rope eviction helper ------------------------------------
            # Head dims are PERMUTED (host-side weight layout) to
            # [rot1, pass_a, rot2, pass_b] so every engine operand starts at
            # a legal partition offset (0/32/64/96): rot pairs are (d, d+32).
            def rope_evict(rp, psrc, dst, nheads, c0, c1_):
                """psrc [64*nheads, w] f32 psum -> dst bf16 with fused rope."""
                w = c1_ - c0
                cs = cosT[:, c0:c1_]
                sn = sinT[:, c0:c1_]
                nc.scalar.activation(
                    dst[0:64 * nheads, :], psrc[0:64 * nheads, :],
                    mybir.ActivationFunctionType.Copy,
                )
                for hb in range(0, 64 * nheads, 64):
                    a = psrc[hb:hb + HALF, :]              # rot1 (start 0/64)
                    b = psrc[hb + 32:hb + 32 + HALF, :]    # rot2 (start 32/96)
                    t1 = rp.tile([HALF, w], f32, tag="rt1", name="t1")
                    t2 = rp.tile([HALF, w], f32, tag="rt2", name="t2")
                    nc.vector.tensor_mul(t1, a, cs)
                    nc.vector.tensor_mul(t2, b, sn)
                    nc.vector.tensor_sub(dst[hb:hb + HALF, :], t1, t2)
                    t3 = rp.tile([HALF, w], f32, tag="rt1", name="t3")
                    t4 = rp.tile([HALF, w], f32, tag="rt2", name="t4")
                    nc.vector.tensor_mul(t3, b, cs)
                    nc.vector.tensor_mul(t4, a, sn)
                    nc.vector.tensor_add(dst[hb + 32:hb + 32 + HALF, :], t3, t4)

            # ================= phase 1: projections =======================
            with (
                tc.tile_pool(name="stage", bufs=2) as stage_pool,
                tc.tile_pool(name="hst", bufs=1) as hst_pool,
                tc.tile_pool(name="wqt", bufs=4) as wqt_pool,
                tc.tile_pool(name="rope", bufs=2) as rp,
                tc.tile_pool(name="p1ps", bufs=1, space="PSUM") as ps1,
                tc.tile_pool(name="tpps", bufs=2, space="PSUM") as ps_tp,
            ):
                wkvT = hst_pool.tile([128, KT, 128], bf16)
                QW = 2048  # staging piece width

                def load_transpose_rows(src_rows, dst_fn):
                    """[128, HID] f32 DRAM rows -> cast bf16 -> PE-transpose
                    (bf16, 4x faster than fp32) -> dst via one DVE copy per
                    4-k-tile group. dst_fn(k0) must be a [128, 4, 128] AP."""
                    for hf in range(HID // QW):
                        stg = stage_pool.tile([128, QW], f32, tag="stg",
                                              bufs=2, name="stg")
                        nc.scalar.dma_start(
                            out=stg, in_=src_rows[:, hf * QW:(hf + 1) * QW]
                        )
                        stgb = stage_pool.tile([128, QW], bf16, tag="stgb",
                                               bufs=2, name="stgb")
                        nc.vector.tensor_copy(stgb, stg)
                        for g in range(QW // 512):
                            k0 = hf * (QW // 128) + g * 4
                            ptp = ps_tp.tile([128, 4, 128], bf16, tag="tp",
                                             name="ptp")
                            for kk in range(4):
                                nc.tensor.transpose(
                                    ptp[:, kk, :],
                                    stgb[:, (g * 4 + kk) * 128:
                                         (g * 4 + kk + 1) * 128],
                                    identity,
                                )
                            nc.vector.tensor_copy(dst_fn(k0), ptp)

                # wkv -> wkvT (once)
                load_transpose_rows(
                    wkv_ext, lambda k0: wkvT[:, k0:k0 + 4, :]
                )

                def emit_wq_material():
                    for k in range(KT):
                        wqs = stage_pool.tile([128, MQ], f32, tag="wqs",
                                              bufs=2, name="wqs")
                        nc.scalar.dma_start(
                            out=wqs, in_=wq_ext[k * 128:(k + 1) * 128, :]
                        )
                        wqc = stage_pool.tile([128, MQ], bf16, tag="wqc",
                                              bufs=2, name="wqc")
                        nc.vector.tensor_copy(wqc, wqs)
                        nc.scalar.dma_start(
                            out=wq_bf[k * 128:(k + 1) * 128, :], in_=wqc
                        )

                MG = 4  # m-tiles per PSUM group
                hsT = None
                for c in range(NCH):
                    c0 = c * CHUNK
                    hsT = hst_pool.tile([128, KT, CHUNK], bf16, tag="hsT")
                    for ts in range(TSUB):
                        r0 = c0 + ts * 128
                        t0_ = ts * 128
                        load_transpose_rows(
                            hs_ext[r0:r0 + 128, :],
                            lambda k0, t0_=t0_: hsT[:, k0:k0 + 4,
                                                    t0_:t0_ + 128],
                        )
                    if c == 0:
                        # emitted after chunk-0 staging so those DMAs win
                        # the priority race; deps still gate the Q matmuls
                        emit_wq_material()

                    # Q projection, m-groups of MG
                    for mg in range(MT // MG):
                        MW = MG * 128
                        psq = [
                            ps1.tile([128, CHUNK], f32, tag=f"psq{m}",
                                     bufs=1, name=f"psq{m}")
                            for m in range(MG)
                        ]
                        for k in range(KT):
                            wqt = wqt_pool.tile([128, MW], bf16, tag="wqt")
                            nc.sync.dma_start(
                                out=wqt,
                                in_=wq_bf[k * 128:(k + 1) * 128,
                                          mg * MW:(mg + 1) * MW],
                            )
                            for m in range(MG):
                                nc.tensor.matmul(
                                    psq[m][:, :],
                                    lhsT=wqt[:, m * 128:(m + 1) * 128],
                                    rhs=hsT[:, k, :],
                                    start=(k == 0), stop=(k == KT - 1),
                                )
                        for m in range(MG):
                            mt = mg * MG + m
                            rope_evict(rp, psq[m], qT[:, mt, c0:c0 + CHUNK],
                                       2, c0, c0 + CHUNK)

                    # K/V projection (k -> partitions 0:64, v -> 64:128)
                    psk = ps1.tile([128, CHUNK], f32, tag="psk", bufs=1)
                    for k in range(KT):
                        nc.tensor.matmul(
                            psk[:, :], lhsT=wkvT[:, k, :], rhs=hsT[:, k, :],
                            start=(k == 0), stop=(k == KT - 1),
                        )
                    rope_evict(rp, psk, kT2[0:64, c0:c0 + CHUNK],
                               1, c0, c0 + CHUNK)
                    nc.sync.dma_start(
                        out=kT2[64:128, c0:c0 + CHUNK],
                        in_=kT2[0:64, c0:c0 + CHUNK],
                    )
                    nc.vector.tensor_copy(vT[:, c0:c0 + CHUNK],
                                          psk[64:128, :])

                    # vext[j-tile] = [128 tokens, D (v dims) + ones column]
                    jt0 = c0 // 128
                    ptv = ps_tp.tile([128, TSUB, D], bf16, tag="tpv",
                                     bufs=1, name="ptv")
                    for jj in range(TSUB):
                        nc.tensor.transpose(
                            ptv[:, jj, :],
                            vT[:, c0 + jj * 128:c0 + (jj + 1) * 128],
                            identity[0:64, 0:64],
                        )
                    nc.vector.tensor_copy(
                        vext[:, jt0:jt0 + TSUB, 0:D], ptv
                    )

            # ================= phase 2: attention =========================
            attnT_ctx = tc.tile_pool(name="attnp", bufs=1)
            ap2 = attnT_ctx.__enter__()
            attnT = ap2.tile([128, MT, T], bf16, name="attnT")
            with (
                tc.tile_pool(name="probs", bufs=6) as probs_pool,
                tc.tile_pool(name="nrm", bufs=2) as nrm_pool,
                tc.tile_pool(name="p2ps", bufs=1, space="PSUM") as ps2,
            ):
                NIC = S // IC
                for b in range(B):
                    for ic in range(NIC):
                        i0 = b * S + ic * IC
                        njt = (ic + 1) * (IC // NJ)
                        for hp in range(NHP):
                            psos = [
                                ps2.tile([128, IC], f32, tag=f"pso{hh}",
                                         bufs=2, name=f"pso{hh}")
                                for hh in range(2)
                            ]
                            # software pipeline: scores for j-tile jt are
                            # emitted before PV of j-tile jt-1 so the exp
                            # (ACT) latency hides under the next matmul
                            pend = []

                            def flush_pv():
                                for (pb_, hh_, n0_, fst, lst) in pend:
                                    nc.tensor.matmul(
                                        psos[hh_][0:D + 1, n0_:IC],
                                        lhsT=vext[:, (b * S) // 128 + pend_jt,
                                                  :],
                                        rhs=pb_[:, n0_:IC],
                                        start=fst, stop=lst,
                                    )
                                pend.clear()

                            for jt in range(njt):
                                j0 = b * S + jt * NJ
                                n0 = max(0, jt * NJ - ic * IC) \
                                    if truncate else 0
                                new = []
                                for hh in range(2):
                                    pss = ps2.tile([128, IC], f32, tag="pss",
                                                   bufs=3, name="pss")
                                    nc.tensor.matmul(
                                        pss[:, n0:IC],
                                        lhsT=kT2[hh * D:(hh + 1) * D,
                                                 j0:j0 + NJ],
                                        rhs=qT[hh * D:(hh + 1) * D, hp,
                                               i0 + n0:i0 + IC],
                                        start=True, stop=True,
                                    )
                                    pb = probs_pool.tile([128, IC], bf16,
                                                         tag="pb", name="pb")
                                    nc.scalar.activation(
                                        pb[:, n0:IC], pss[:, n0:IC],
                                        mybir.ActivationFunctionType.Exp,
                                        scale=SCALE,
                                    )
                                    if jt * NJ >= ic * IC:  # diagonal block
                                        if truncate:
                                            nc.vector.tensor_mul(
                                                pb[:, n0:n0 + NJ],
                                                pb[:, n0:n0 + NJ], tri_mask
                                            )
                                        else:
                                            di = (jt * NJ - ic * IC) // NJ
                                            nc.vector.tensor_mul(
                                                pb, pb, full_masks[di]
                                            )
                                    new.append((pb, hh, n0, jt == 0,
                                                jt == njt - 1))
                                if pipelined:
                                    pend_jt = jt - 1
                                    flush_pv()
                                    pend = new
                                else:
                                    pend = new
                                    pend_jt = jt
                                    flush_pv()
                            if pipelined:
                                pend_jt = njt - 1
                                flush_pv()

                            for hh in range(2):
                                rc = nrm_pool.tile([1, IC], f32, tag="rc",
                                                   name="rc")
                                nc.vector.reciprocal(
                                    rc, psos[hh][D:D + 1, :]
                                )
                                rcb = nrm_pool.tile([D, IC], f32, tag="rcb",
                                                    name="rcb")
                                nc.gpsimd.partition_broadcast(rcb, rc)
                                nc.vector.tensor_mul(
                                    attnT[hh * D:(hh + 1) * D, hp,
                                          i0:i0 + IC],
                                    psos[hh][0:D, :], rcb,
                                )

            # ================= phase 3: output projection + RS ============
            with (
                tc.tile_pool(name="wot", bufs=2) as wot_pool,
                tc.tile_pool(name="pout", bufs=4) as pout_pool,
                tc.tile_pool(name="p3ps", bufs=2, space="PSUM") as ps3,
            ):
                for ci, (b, o0, w) in enumerate(rs_widths):
                    for oci in range(w // 512):
                        oc0 = o0 + oci * 512
                        ws = []
                        for a in range(MT):
                            wf = wot_pool.tile([128, 512], f32,
                                               tag=f"wof{a}", name=f"wof{a}")
                            nc.sync.dma_start(
                                out=wf,
                                in_=wo_ext[a * 128:(a + 1) * 128,
                                           oc0:oc0 + 512],
                            )
                            wt = wot_pool.tile([128, 512], bf16,
                                               tag=f"wot{a}", name=f"wot{a}")
                            nc.vector.tensor_copy(wt, wf)
                            ws.append(wt)
                        for t in range(S // 128):
                            tt = b * (S // 128) + t
                            ps = ps3.tile([128, 512], f32, tag="pso3")
                            for a in range(MT):
                                nc.tensor.matmul(
                                    ps[:, :],
                                    lhsT=attnT[:, a, tt * 128:(tt + 1) * 128],
                                    rhs=ws[a],
                                    start=(a == 0), stop=(a == MT - 1),
                                )
                            ob = pout_pool.tile([128, 512], bf16, tag="ob")
                            if t % 2 == 0:
                                nc.scalar.activation(
                                    ob, ps, mybir.ActivationFunctionType.Copy
                                )
                            else:
                                nc.vector.tensor_copy(ob, ps)
                            nc.sync.dma_start(
                                out=partials[ci][t * 128:(t + 1) * 128,
                                                 oci * 512:(oci + 1) * 512],
                                in_=ob,
                            )
                    nc.gpsimd.collective_compute(
                        "ReduceScatter",
                        mybir.AluOpType.add,
                        ins=[partials[ci][:, :].opt()],
                        outs=[rs_outs[ci][:, :].opt()],
                        replica_groups=groups,
                    )
                # final copies after all collectives are triggered, so no
                # compute ever queues behind a collective-completion wait
                SO = S // cfg.n_cores  # output rows per core per batch (128)
                for ci, (b, o0, w) in enumerate(rs_widths):
                    nc.gpsimd.dma_start(
                        out=out_ext[b * SO:(b + 1) * SO, o0:o0 + w],
                        in_=rs_outs[ci][:, :],
                    )
            attnT_ctx.__exit__(None, None, None)

    nc.compile()
    return nc


# ---------------------------------------------------------------- host side

def shard_inputs(cfg: Cfg, position_ids, hidden_states, Wq, Wkv, Wo):
    """Full inputs -> per-core input maps (slicing/layout/dtype only)."""
    hs = np.ascontiguousarray(np.asarray(hidden_states, dtype=np.float32))
    pos = np.asarray(position_ids).astype(np.float32).reshape(1, cfg.T)
    Wq = np.asarray(Wq, dtype=np.float32)
    Wkv = np.asarray(Wkv, dtype=np.float32)
    Wo = np.asarray(Wo, dtype=np.float32)
    D = cfg.D
    HKV = cfg.n_cores
    half = cfg.RD // 2
    # permuted head-dim order [rot1, pass_a, rot2, pass_b]: rope pairs land
    # at partition offsets (d, d+32), which the engines can address
    perm = np.concatenate([
        np.arange(0, half),
        np.arange(2 * half, 3 * half),
        np.arange(half, 2 * half),
        np.arange(3 * half, D),
    ])
    in_maps = []
    for c in range(cfg.n_cores):
        wq_c = Wq[c * cfg.MQ:(c + 1) * cfg.MQ, :]
        wq_c = wq_c.reshape(-1, D, cfg.HID)[:, perm, :].reshape(cfg.MQ, cfg.HID)
        wqT_c = np.ascontiguousarray(wq_c.T)
        wk_c = Wkv[c * D:(c + 1) * D, :][perm, :]
        wv_c = Wkv[HKV * D + c * D:HKV * D + (c + 1) * D, :]
        wkv_c = np.ascontiguousarray(np.concatenate([wk_c, wv_c], axis=0))
        woT_c = np.ascontiguousarray(Wo[:, c * cfg.MQ:(c + 1) * cfg.MQ].T)
        in_maps.append(
            {"hs": hs, "wqT": wqT_c, "wkv": wkv_c, "woT": woT_c, "pos": pos}
        )
    return in_maps


_NC_CACHE = {}
_BUILD_KW = {}


def _get_nc(cfg: Cfg):
    key = (cfg, tuple(sorted(_BUILD_KW.items())))
    if key not in _NC_CACHE:
        _NC_CACHE[key] = build_nc(cfg, **_BUILD_KW)
    return _NC_CACHE[key]


def run_on_hw(cfg: Cfg, in_maps, trace=False):
    from concourse.bass_utils import run_bass_kernel_spmd

    nc = _get_nc(cfg)
    res = run_bass_kernel_spmd(
        nc, in_maps, core_ids=list(range(cfg.n_cores)), trace=trace
    )
    # core c's out rows [b*SO:(b+1)*SO] hold global tokens
    # [b*S + c*SO, b*S + (c+1)*SO)
    SO = cfg.S // cfg.n_cores
    out = np.empty((cfg.T, cfg.HID), dtype=np.float32)
    for c in range(cfg.n_cores):
        r = np.asarray(res.results[c]["out"], dtype=np.float32)
        for b in range(cfg.B):
            out[b * cfg.S + c * SO:b * cfg.S + (c + 1) * SO] = \
                r[b * SO:(b + 1) * SO]
    return out, res


def kernel(position_ids, hidden_states, Wq, Wkv, Wo, num_seqs):
    cfg = FULL
    in_maps = shard_inputs(cfg, position_ids, hidden_states, Wq, Wkv, Wo)
    out, _ = run_on_hw(cfg, in_maps, trace=False)
    return out
